# revision 105
# baseline (speedup 1.0000x reference)
"""Trainium2 Bass kernel for nn_Axial_PFCU_Continuous (dense_cnn).

Math (per sample, C=96, H=W=128), folded host-side:
  z     = Wf~ @ (m+l) + anchor + bz
        = sum over 17 shift terms A_d @ shift_d(x)   (+ bias on evict)
  pre   = PReLU(z, a)
  coord attention: spatial means of pre -> tiny matmuls -> sigmoid gates
  out   = pre * ah(c,h) * aw(c,w)

Sharding: pure data-parallel, 1 of 8 batch samples per NeuronCore.

Key trick: all 16 shifted terms run on the PE as fp8-e4m3 DoubleRow
matmuls — each pass packs TWO shift offsets into the rhs "pair" dim via
a strided AP on zero-padded fp8 copies of x (H-padded row-major for
H-shifts; W-padded chunk-transposed for W-shifts), so one 512-col pass
costs 256 PE cycles and edge handling is free. Only the large center
term (d=0) runs in bf16 for accuracy. The bias rides the PReLU evict.

Per core engine budget: PE ~41us (z passes + xw ident), ACT ~22us
(PSUM evict + coord chain), DVE ~20us (xh reduces, half the gate
multiplies), Pool ~14us (other half), DMA ~29us.
"""
import sys
import math

sys.path.insert(0, '/opt/trn_rl_repo')

import numpy as np
import ml_dtypes
from contextlib import ExitStack

import concourse.bass as bass
import concourse.bacc as bacc
from concourse import mybir, tile
from concourse.bass_utils import run_bass_kernel_spmd

f32 = mybir.dt.float32
bf16 = mybir.dt.bfloat16
f8 = mybir.dt.float8e4
ALU = mybir.AluOpType
AF = mybir.ActivationFunctionType
DR = mybir.MatmulPerfMode.DoubleRow

B, C, H, W = 8, 96, 128, 128
HW = H * W
EPS = 1e-5
N_CORES = 8

PADH = 16            # zero-pad rows each side in x_f8h
HP = H + 2 * PADH    # 160
PADW = 16
WP = W + 2 * PADW    # 160

NCHUNK = 32          # 4-row z chunks
CH = H // NCHUNK     # 4 rows per chunk
CW = CH * W          # 512 cols per chunk
NBLK = 16            # xh/finals blocks (8 rows)
BH = H // NBLK
NGRP = 8             # ah gate groups (2 blocks each)

# (d0, d1) shift pairs per axis; each pair = one DoubleRow pass
PAIRS = ((-1, 1), (-4, 4), (-8, 8), (-16, 16))
SSTAT = 8.0          # z stationary pre-scale (undone on the evict)

_GRAPH_CACHE = {}


# ----------------------------------------------------------------- host folds
def _taps(w_taps, r):
    """offset -> (C,) coefficient for the integer-shift decomposition."""
    r = max(float(r), 1.0)
    K = w_taps.shape[1]
    d2w = {}
    for i in range(K):
        s = (i - K // 2) * r
        f = math.floor(s)
        frac = s - f
        for d, wt in ((int(f), 1.0 - frac), (int(f) + 1, frac)):
            if wt != 0.0:
                if d not in d2w:
                    d2w[d] = np.zeros(C, np.float64)
                d2w[d] = d2w[d] + wt * np.asarray(w_taps[:, i], np.float64)
    return {d: w for d, w in d2w.items() if abs(d) < H}


def _merge(a, b):
    out = dict(a)
    for d, w in b.items():
        out[d] = out.get(d, np.zeros(C, np.float64)) + w
    return out


class _Pack:
    def __init__(self):
        self.cols = {}
        self.parts = []
        self.pos = 0

    def put(self, name, arr):
        arr = np.asarray(arr, np.float64)
        if arr.ndim == 1:
            arr = arr[:, None]
        pad = np.zeros((C, arr.shape[1]), np.float64)
        pad[:arr.shape[0], :] = arr
        self.cols[name] = (self.pos, arr.shape[1])
        self.parts.append(pad)
        self.pos += arr.shape[1]

    def done(self, dt):
        return np.concatenate(self.parts, axis=1).astype(dt)


def _fold(inp):
    g = lambda k: np.asarray(inp[k], np.float64)
    hA = _merge(_taps(g('wh_m'), float(np.asarray(inp['r_m']))),
                _taps(g('wh_l'), float(np.asarray(inp['r_l']))))
    wA = _merge(_taps(g('ww_m'), float(np.asarray(inp['r_m']))),
                _taps(g('ww_l'), float(np.asarray(inp['r_l']))))
    hA[0] = hA.get(0, np.zeros(C)) + 2.0    # identity terms of m+l
    wA.setdefault(0, np.zeros(C))
    offs = sorted(d for d in set(hA) | set(wA) if d != 0)
    assert all(abs(d) <= PADH for d in offs), f"shift exceeds pad: {offs}"
    pair_ds = {d for p in PAIRS for d in p}
    assert set(offs) <= pair_ds, f"unpaired shifts: {set(offs) - pair_ds}"

    sf = g('bnf_g') / np.sqrt(g('bnf_v') + EPS)
    wfuse_t = (g('w_fuse') * sf[:, None]).T.copy()      # (Cin, Cout) lhsT
    bf = g('bnf_b') - g('bnf_m') * sf

    ds = g('dg_g') / np.sqrt(g('dg_v') + EPS)
    db = g('dg_b') - g('dg_m') * ds
    dg_wh, dg_ww = g('dg_wh'), g('dg_ww')
    anchH = {-1: ds * dg_wh[:, 0], 1: ds * dg_wh[:, 2]}
    anchW = {-1: ds * dg_ww[:, 0], 1: ds * dg_ww[:, 2]}
    cB0 = ds * (dg_wh[:, 1] + dg_ww[:, 1] + 1.0)

    cs = g('ca_g') / np.sqrt(g('ca_v') + EPS)
    cb = g('ca_b') - g('ca_m') * cs

    def lhsT_for(axis, d):
        """stationary (lhsT layout, (Cin, Cout)) for one shift term."""
        A = hA if axis == 'h' else wA
        anch = anchH if axis == 'h' else anchW
        m = np.zeros((C, C))
        if d in A:
            m = m + wfuse_t * A[d][:, None]
        if d in anch:
            m = m + np.diag(anch[d])
        return m

    # f32 consts
    pkf = _Pack()
    pkf.put('bz', bf + db)
    pkf.put('act_a', g('act_a'))
    pkf.put('zero', np.zeros(C))
    pkf.put('one', np.ones(C))
    pkf.put('cas', cs)
    pkf.put('cab', cb)
    pkf.put('caa', g('ca_a'))
    consts = pkf.done(np.float32)

    # bf16 consts: center stationary + identity (xw accumulate)
    pkb = _Pack()
    pkb.put('Acen', (wfuse_t * (hA[0] + wA[0])[:, None] + np.diag(cB0))
            * SSTAT)
    pkb.put('ident', np.eye(C))
    pkb.put('caw1b', (g('ca_w1') / float(W)).T)   # (C, 8); 1/W mean fold
    pkb.put('cawhb', g('ca_wh').T)                # (8, C)
    pkb.put('cawwb', g('ca_ww').T)
    constb = pkb.done(ml_dtypes.bfloat16)

    # fp8 consts: DoubleRow pair blocks (C, 2*C) = [slot0 | slot1].
    # All z stationaries are pre-scaled by SSTAT (power of 2) to pull the
    # small fp8 entries out of e4m3's subnormal range; the PReLU evict
    # applies scale=1/SSTAT to undo it.
    pk8 = _Pack()
    for axis in ('h', 'w'):
        for d0, d1 in PAIRS:
            blk = np.concatenate([lhsT_for(axis, d0), lhsT_for(axis, d1)], 1)
            pk8.put(f'P{axis}{d0}_{d1}', blk * SSTAT)
    const8 = pk8.done(ml_dtypes.float8_e4m3fn)

    key = (consts.shape[1], constb.shape[1], const8.shape[1])
    return consts, pkf.cols, constb, pkb.cols, const8, pk8.cols, key


def _prep_x(x):
    """Per-sample input tiles: bf16 flat, fp8 H-padded, fp8 W-padded
    chunk-transposed."""
    xb = x.astype(ml_dtypes.bfloat16).reshape(B, C, HW)
    x8 = x.astype(ml_dtypes.float8_e4m3fn)
    f8h = np.zeros((B, C, HP, W), ml_dtypes.float8_e4m3fn)
    f8h[:, :, PADH:PADH + H, :] = x8
    # x_f8w[c, k, w', j] = x[c, 4k+j, w'-PADW]
    f8w = np.zeros((B, C, NCHUNK, WP, CH), ml_dtypes.float8_e4m3fn)
    xr = x8.reshape(B, C, NCHUNK, CH, W)
    f8w[:, :, :, PADW:PADW + W, :] = np.swapaxes(xr, 3, 4)
    return (xb, f8h.reshape(B, C, HP * W), f8w.reshape(B, C, NCHUNK * WP * CH))


# -------------------------------------------------------------- graph builder
def _build(colf, colb, col8, ckf, ckb, ck8):
    nc = bacc.Bacc()
    xb_p = nc.declare_dram_parameter("xb", (C, HW), bf16, isOutput=False)
    xh_p = nc.declare_dram_parameter("xf8h", (C, HP * W), f8, isOutput=False)
    xw_p = nc.declare_dram_parameter("xf8w", (C, NCHUNK * WP * CH), f8,
                                     isOutput=False)
    cf_p = nc.declare_dram_parameter("consts", (C, ckf), f32, isOutput=False)
    cb_p = nc.declare_dram_parameter("constb", (C, ckb), bf16, isOutput=False)
    c8_p = nc.declare_dram_parameter("const8", (C, ck8), f8, isOutput=False)
    o_p = nc.declare_dram_parameter("out", (C, HW), bf16, isOutput=True)

    with tile.TileContext(nc) as tc, ExitStack() as ctx:
        big = ctx.enter_context(tc.tile_pool(name="big", bufs=1))
        tpool = ctx.enter_context(tc.tile_pool(name="tpool", bufs=3))
        psq = ctx.enter_context(tc.tile_pool(name="psq", bufs=4, space="PSUM"))
        psa = ctx.enter_context(tc.tile_pool(name="psa", bufs=1, space="PSUM"))
        pss = ctx.enter_context(tc.tile_pool(name="pss", bufs=2, space="PSUM"))

        cbt = big.tile([C, ckb], bf16, tag="cbt")
        nc.sync.dma_start(cbt[:], cb_p[:])
        c8t = big.tile([C, ck8], f8, tag="c8t")
        nc.sync.dma_start(c8t[:], c8_p[:])
        cst = big.tile([C, ckf], f32, tag="cst")
        nc.scalar.dma_start(cst[:], cf_p[:])

        def cc(name, i=0):
            p0, n = colf[name]
            assert i < n
            return cst[:, p0 + i:p0 + i + 1]

        def crf(name, rows=C):
            p0, n = colf[name]
            return cst[0:rows, p0:p0 + n]

        def cbr(name):
            p0, n = colb[name]
            return cbt[0:C, p0:p0 + n]

        def c8pair(name):
            p0, n = col8[name]
            return c8t[0:C, p0:p0 + n].rearrange("p (t m) -> p t m", t=2)

        # input tiles + chunked DMA, interleaved so chunk 0 deps land first
        xb_t = big.tile([C, HW], bf16, tag="xb")
        xh_t = big.tile([C, HP * W], f8, tag="xf8h")
        xw_t = big.tile([C, NCHUNK * WP * CH], f8, tag="xf8w")
        pre = big.tile([C, HW], bf16, tag="pre")
        pre3 = pre[:].rearrange("p (h w) -> p h w", w=W)

        # chunked input DMA: few slabs (SP seq cost ~0.6us per dma_start),
        # first slab of each tile small so chunk 0 can start early
        SLW = WP * CH                       # 640 cols per xf8w chunk
        def slabs(total, first, step):
            cuts = [0, first]
            while cuts[-1] < total:
                cuts.append(min(total, cuts[-1] + step))
            return list(zip(cuts[:-1], cuts[1:]))
        dma_order = []
        for (a, b) in slabs(NCHUNK, 4, 7):
            dma_order.append(('w', a * SLW, b * SLW))
        for (a, b) in slabs(HP // 8, 5, 5):
            dma_order.append(('h', a * 8 * W, b * 8 * W))
        for (a, b) in slabs(NBLK, 2, 4):
            dma_order.append(('b', a * BH * W, b * BH * W))
        # chunk-0 deps first: W slab (W-passes run first), then H halo
        pri = {'w': 0, 'h': 1, 'b': 2}
        dma_order.sort(key=lambda t: (t[1], pri[t[0]]))
        # split input streams across queues: each dma_start holds its
        # queue's sequencer ~0.6-2us, so one queue serializes the fill.
        # SP: fp8 tiles; Pool (SWDGE, idle early): the bf16 center tile.
        for kind, a, b in dma_order:
            t, p = {'w': (xw_t, xw_p), 'h': (xh_t, xh_p),
                    'b': (xb_t, xb_p)}[kind]
            q = nc.gpsimd if kind == 'b' else nc.sync
            q.dma_start(t[:, a:b], p[:, a:b])

        zcol = cc('zero')
        # engine warmups: ACT tables, small copies, PE clock ramp.
        # The PE warmers run on a memset scratch (no DMA dependency) so the
        # PE is busy from ~0.5us through the DMA fill — the cost model
        # resets the PE clock ramp after any idle gap.
        wsc = big.tile([C, 256], bf16, tag="wsc")
        nc.vector.memset(wsc[:], 0.0)
        wrm = big.tile([C, 4], f32, tag="wrm")
        nc.scalar.activation(wrm[:, 0:1], zcol, AF.Prelu, bias=zcol, scale=1.0,
                             alpha=cc('act_a'))
        nc.scalar.activation(wrm[:, 3:4], zcol, AF.Sigmoid, bias=zcol,
                             scale=1.0)
        nc.gpsimd.tensor_copy(wrm[:, 2:3], zcol)
        pwm = pss.tile([C, 256], f32, tag="small")
        for wi in range(10):
            nc.tensor.matmul(pwm[:], wsc[:, 0:96], wsc[:],
                             start=(wi == 0), stop=(wi == 9))

        def h_rhs(k, d0, d1):
            fullap = xh_t[:]
            return bass.AP(
                tensor=fullap.tensor,
                offset=fullap.offset + (PADH + k * CH + d0) * W,
                ap=[fullap.ap[0], [(d1 - d0) * W, 2], [1, CW]])

        def w_rhs(k, d0, d1):
            fullap = xw_t[:]
            return bass.AP(
                tensor=fullap.tensor,
                offset=fullap.offset + k * SLW + (PADW + d0) * CH,
                ap=[fullap.ap[0], [(d1 - d0) * CH, 2], [CH, W], [1, CH]])

        # xw column-sum accumulator (psum), fed by deferred ident matmuls
        xwp = psa.tile([C, CH, W], f32, tag="xwp")
        yinh = big.tile([C, H], bf16, tag="yinh")   # bf16: keeps reduce at 2x
        yinw = big.tile([C, W], bf16, tag="yinw")
        # even chunks 0-22 accumulate xw on DVE (hidden in mid-loop gaps)
        # instead of PE ident matmuls; folded once at k=26
        NDVE_XW = 14
        ywd = big.tile([C, W * NDVE_XW], bf16, tag="ywd")
        ywd3 = ywd[:].rearrange("p (w k) -> p w k", k=NDVE_XW)
        yw2a = big.tile([C, W], bf16, tag="yw2a")
        # odd chunks 1-23 accumulate xw on Pool via row-fold chains
        accw = big.tile([C, W], bf16, tag="accw")
        ah = big.tile([C, H], bf16, tag="ah")
        aw = big.tile([C, W], bf16, tag="aw")
        y2 = big.tile([8, 2 * H], bf16, tag="y2")
        stage_t = [big.tile([C, BH * W], bf16, tag=f"st{b}", name=f"st{b}")
                   for b in range(NBLK)]

        deferred = []   # (chunk, pre-slice AP) for xw ident matmuls
        gcnt = [0]
        NPE_XW = NCHUNK - 2 * NDVE_XW

        def emit_xw(k, sl):
            nc.tensor.matmul(xwp[:], cbr('ident'), sl,
                             start=(gcnt[0] == 0),
                             stop=(gcnt[0] == NPE_XW - 1))
            gcnt[0] += 1

        # hoist the first center passes: they only need the first xb
        # slab, so they fill the PE while the fp8 halo slabs stream in
        pk_hoist = []
        for k in range(3):
            pk = psq.tile([C, CH, W], f32, tag="pk", name=f"pkh{k}")
            pk_hoist.append(pk)
            nc.tensor.matmul(pk[:].rearrange("p r w -> p (r w)"), cbr('Acen'),
                             xb_t[:, k * CW:(k + 1) * CW],
                             start=True, stop=False)

        for k in range(NCHUNK):
            if k < 3:
                pk = pk_hoist[k]
            else:
                pk = psq.tile([C, CH, W], f32, tag="pk")
            pkf = pk[:].rearrange("p r w -> p (r w)")
            if k >= 3:
                nc.tensor.matmul(pkf, cbr('Acen'),
                                 xb_t[:, k * CW:(k + 1) * CW],
                                 start=True, stop=False)
            pkt = pk[:].rearrange("p r w -> p w r")
            for d0, d1 in PAIRS:
                nc.tensor.matmul(pkt, c8pair(f'Pw{d0}_{d1}'), w_rhs(k, d0, d1),
                                 start=False, stop=False, perf_mode=DR)
            for i, (d0, d1) in enumerate(PAIRS):
                nc.tensor.matmul(pkf, c8pair(f'Ph{d0}_{d1}'), h_rhs(k, d0, d1),
                                 start=False, stop=(i == len(PAIRS) - 1),
                                 perf_mode=DR)
            # evict: pre = prelu(z/SSTAT + bz)
            nc.scalar.activation(pre[:, k * CW:(k + 1) * CW], pkf,
                                 AF.Prelu, bias=cc('bz'), scale=1.0 / SSTAT,
                                 alpha=cc('act_a'))
            sl = pre3[:, k * CH:(k + 1) * CH, :]
            if k >= NCHUNK - 2:
                # last block's xh as per-chunk reduces: the chunk-30 half
                # runs off-critical; only 593ns rides the tail
                with nc.allow_low_precision(reason="xh sums feed gates"):
                    nc.vector.tensor_reduce(yinh[:, k * CH:(k + 1) * CH],
                                            sl, axis=mybir.AxisListType.X,
                                            op=ALU.add)
            if k % 2 == 0 and k < 2 * NDVE_XW:
                with nc.allow_low_precision(reason="xw partial sums feed "
                                            "sigmoid gates"):
                    nc.vector.tensor_reduce(
                        ywd3[:, :, k // 2:k // 2 + 1].squeeze(2),
                        sl.rearrange("p h w -> p w h"),
                        axis=mybir.AxisListType.X, op=ALU.add)
            elif k % 2 == 1 and k < 2 * NDVE_XW:
                t1 = tpool.tile([C, 2 * W], bf16, tag="xf")
                t13 = t1[:].rearrange("p (h w) -> p h w", w=W)
                nc.gpsimd.tensor_tensor(t13, sl[:, 0:2, :], sl[:, 2:4, :],
                                        op=ALU.add)
                t1a = t13[:, 0:1, :].squeeze(1)
                t1b = t13[:, 1:2, :].squeeze(1)
                if k == 1:
                    nc.gpsimd.tensor_tensor(accw[:], t1a, t1b, op=ALU.add)
                else:
                    t2 = tpool.tile([C, W], bf16, tag="xf2")
                    nc.gpsimd.tensor_tensor(t2[:], t1a, t1b, op=ALU.add)
                    nc.gpsimd.tensor_tensor(accw[:], accw[:], t2[:],
                                            op=ALU.add)
            else:
                deferred.append((k, sl))
                if len(deferred) >= 3:
                    emit_xw(*deferred.pop(0))
            if k == 2 * NDVE_XW + 2:
                with nc.allow_low_precision(reason="xw partials"):
                    nc.vector.tensor_reduce(yw2a[:], ywd3,
                                            axis=mybir.AxisListType.X,
                                            op=ALU.add)

            if k % 2 == 1:
                # xh row sums: fold W in half on Pool (522ns), half again on
                # DVE (193ns), small reduce on DVE (327ns) — a direct DVE
                # reduce is 1127ns (tensor_reduce never gets a perf mode).
                # The last block reduces directly on DVE: the fold chain's
                # Pool hop would sit on the critical tail.
                b = k // 2   # finished block
                r0, r1 = b * BH, (b + 1) * BH
                with nc.allow_low_precision(reason="xh sums feed sigmoid "
                                            "gates"):
                    if b == NBLK - 1:
                        pass   # handled per-chunk below
                    else:
                        fold1 = tpool.tile([C, BH * (W // 2)], bf16,
                                           tag="f1")
                        f13 = fold1[:].rearrange("p (h w) -> p h w",
                                                 w=W // 2)
                        nc.gpsimd.tensor_tensor(f13,
                                                pre3[:, r0:r1, 0:W // 2],
                                                pre3[:, r0:r1, W // 2:W],
                                                op=ALU.add)
                        fold2 = tpool.tile([C, BH * (W // 4)], bf16,
                                           tag="f2")
                        f23 = fold2[:].rearrange("p (h w) -> p h w",
                                                 w=W // 4)
                        nc.vector.tensor_tensor(f23, f13[:, :, 0:W // 4],
                                                f13[:, :, W // 4:W // 2],
                                                op=ALU.add)
                        nc.vector.tensor_reduce(yinh[:, r0:r1], f23,
                                                axis=mybir.AxisListType.X,
                                                op=ALU.add)
            # gate groups: pairs of blocks, except the last two blocks get
            # their own groups so only block 15's chain rides the tail
            G_END = {3: (0, 2), 7: (2, 4), 11: (4, 6), 15: (6, 8),
                     19: (8, 10), 23: (10, 12), 27: (12, 14),
                     29: (14, 15), 31: (15, 16)}
            if k in G_END:
                b0g, b1g = G_END[k]
                c0, c1 = b0g * BH, b1g * BH   # h cols
                y1p = pss.tile([8, c1 - c0], f32, tag="small")
                nc.tensor.matmul(y1p[:], cbr('caw1b')[:, 0:8], yinh[:, c0:c1],
                                 start=True, stop=True)
                nc.scalar.activation(y2[:, c0:c1], y1p[:], AF.Prelu,
                                     bias=cc('cab')[0:8, :],
                                     scale=cc('cas')[0:8, :],
                                     alpha=cc('caa')[0:8, :])
                ahp = pss.tile([C, c1 - c0], f32, tag="small")
                nc.tensor.matmul(ahp[:], cbr('cawhb')[0:8, :], y2[:, c0:c1],
                                 start=True, stop=True)
                nc.scalar.activation(ah[:, c0:c1], ahp[:], AF.Sigmoid,
                                     bias=zcol, scale=1.0)
                # stage: o = pre * ah (bcast w) into separate tiles so the
                # xw matmuls keep reading a clean pre
                for b in range(b0g, b1g):
                    r0, r1 = b * BH, (b + 1) * BH
                    ah_b = ah[:, r0:r1].unsqueeze(2).broadcast_to((C, BH, W))
                    o3 = stage_t[b][:].rearrange("p (h w) -> p h w", w=W)
                    # ah bcast has stride-0 innermost -> 1x on DVE, so Pool
                    # (853ns) takes most stages
                    eng = nc.vector if b % 4 == 3 else nc.gpsimd
                    eng.tensor_tensor(o3, pre3[:, r0:r1, :], ah_b,
                                      op=ALU.mult)

        for item in deferred:
            emit_xw(*item)

        # aw tail: fold xw psum, add the DVE partial, tiny chain
        with nc.allow_low_precision(reason="xw sums feed sigmoid gates"):
            nc.vector.tensor_reduce(yinw[:],
                                    xwp[:].rearrange("p j w -> p w j"),
                                    axis=mybir.AxisListType.X, op=ALU.add)
        nc.vector.tensor_tensor(yinw[:], yinw[:], yw2a[:], op=ALU.add)
        nc.vector.tensor_tensor(yinw[:], yinw[:], accw[:], op=ALU.add)
        y1w = pss.tile([8, H], f32, tag="small")
        nc.tensor.matmul(y1w[:], cbr('caw1b')[:, 0:8], yinw[:],
                         start=True, stop=True)
        nc.scalar.activation(y2[:, H:2 * H], y1w[:], AF.Prelu,
                             bias=cc('cab')[0:8, :], scale=cc('cas')[0:8, :],
                             alpha=cc('caa')[0:8, :])
        awp = pss.tile([C, W], f32, tag="small")
        nc.tensor.matmul(awp[:], cbr('cawwb')[0:8, :], y2[:, H:2 * H],
                         start=True, stop=True)
        nc.scalar.activation(aw[:], awp[:], AF.Sigmoid, bias=zcol, scale=1.0)

        # finals: out = stage * aw (bcast h; packed innermost everywhere ->
        # STT can hit the DVE 4x mode). Out-DMAs alternate idle SP/ACT.
        aw_b = aw[:].unsqueeze(1).broadcast_to((C, BH, W))
        for b in range(NBLK):
            r0, r1 = b * BH, (b + 1) * BH
            o3 = stage_t[b][:].rearrange("p (h w) -> p h w", w=W)
            eng = nc.gpsimd if b % 16 in (2, 5, 8, 11, 14) else nc.vector
            eng.tensor_tensor(o3, o3, aw_b, op=ALU.mult)
            dq = nc.sync if b % 2 == 0 else nc.scalar
            dq.dma_start(o_p[:, r0 * W:r1 * W], stage_t[b][:])

    nc.compile()
    return nc


def _get_graph(key, colf, colb, col8, ckf, ckb, ck8):
    if key not in _GRAPH_CACHE:
        _GRAPH_CACHE[key] = _build(colf, colb, col8, ckf, ckb, ck8)
    return _GRAPH_CACHE[key]


# ------------------------------------------------------------------ interface
def _run(inputs, trace=False):
    x = np.ascontiguousarray(np.asarray(inputs['x'], np.float32))
    assert x.shape == (B, C, H, W)
    consts, colf, constb, colb, const8, col8, key = _fold(inputs)
    nc = _get_graph(key, colf, colb, col8,
                    consts.shape[1], constb.shape[1], const8.shape[1])
    xb, f8h, f8w = _prep_x(x)
    in_maps = []
    for i in range(N_CORES):
        in_maps.append({'xb': np.ascontiguousarray(xb[i]),
                        'xf8h': np.ascontiguousarray(f8h[i]),
                        'xf8w': np.ascontiguousarray(f8w[i]),
                        'consts': consts, 'constb': constb,
                        'const8': const8})
    res = run_bass_kernel_spmd(nc, in_maps, list(range(N_CORES)), trace=trace)
    out = np.stack([res.results[i]['out'].astype(np.float32).reshape(C, H, W)
                    for i in range(N_CORES)], axis=0)
    return out, res


def kernel(**inputs):
    out, _ = _run(inputs, trace=False)
    return out


# revision 106
# speedup vs baseline: 1.0010x; 1.0010x over previous
"""Trainium2 Bass kernel for nn_Axial_PFCU_Continuous (dense_cnn).

Math (per sample, C=96, H=W=128), folded host-side:
  z     = Wf~ @ (m+l) + anchor + bz
        = sum over 17 shift terms A_d @ shift_d(x)   (+ bias on evict)
  pre   = PReLU(z, a)
  coord attention: spatial means of pre -> tiny matmuls -> sigmoid gates
  out   = pre * ah(c,h) * aw(c,w)

Sharding: pure data-parallel, 1 of 8 batch samples per NeuronCore.

Key trick: all 16 shifted terms run on the PE as fp8-e4m3 DoubleRow
matmuls — each pass packs TWO shift offsets into the rhs "pair" dim via
a strided AP on zero-padded fp8 copies of x (H-padded row-major for
H-shifts; W-padded chunk-transposed for W-shifts), so one 512-col pass
costs 256 PE cycles and edge handling is free. Only the large center
term (d=0) runs in bf16 for accuracy. The bias rides the PReLU evict.

Per core engine budget: PE ~41us (z passes + xw ident), ACT ~22us
(PSUM evict + coord chain), DVE ~20us (xh reduces, half the gate
multiplies), Pool ~14us (other half), DMA ~29us.
"""
import sys
import math

sys.path.insert(0, '/opt/trn_rl_repo')

import numpy as np
import ml_dtypes
from contextlib import ExitStack

import concourse.bass as bass
import concourse.bacc as bacc
from concourse import mybir, tile
from concourse.bass_utils import run_bass_kernel_spmd

f32 = mybir.dt.float32
bf16 = mybir.dt.bfloat16
f8 = mybir.dt.float8e4
ALU = mybir.AluOpType
AF = mybir.ActivationFunctionType
DR = mybir.MatmulPerfMode.DoubleRow

B, C, H, W = 8, 96, 128, 128
HW = H * W
EPS = 1e-5
N_CORES = 8

PADH = 16            # zero-pad rows each side in x_f8h
HP = H + 2 * PADH    # 160
PADW = 16
WP = W + 2 * PADW    # 160

NCHUNK = 32          # 4-row z chunks
CH = H // NCHUNK     # 4 rows per chunk
CW = CH * W          # 512 cols per chunk
NBLK = 16            # xh/finals blocks (8 rows)
BH = H // NBLK
NGRP = 8             # ah gate groups (2 blocks each)

# (d0, d1) shift pairs per axis; each pair = one DoubleRow pass
PAIRS = ((-1, 1), (-4, 4), (-8, 8), (-16, 16))
SSTAT = 8.0          # z stationary pre-scale (undone on the evict)

_GRAPH_CACHE = {}


# ----------------------------------------------------------------- host folds
def _taps(w_taps, r):
    """offset -> (C,) coefficient for the integer-shift decomposition."""
    r = max(float(r), 1.0)
    K = w_taps.shape[1]
    d2w = {}
    for i in range(K):
        s = (i - K // 2) * r
        f = math.floor(s)
        frac = s - f
        for d, wt in ((int(f), 1.0 - frac), (int(f) + 1, frac)):
            if wt != 0.0:
                if d not in d2w:
                    d2w[d] = np.zeros(C, np.float64)
                d2w[d] = d2w[d] + wt * np.asarray(w_taps[:, i], np.float64)
    return {d: w for d, w in d2w.items() if abs(d) < H}


def _merge(a, b):
    out = dict(a)
    for d, w in b.items():
        out[d] = out.get(d, np.zeros(C, np.float64)) + w
    return out


class _Pack:
    def __init__(self):
        self.cols = {}
        self.parts = []
        self.pos = 0

    def put(self, name, arr):
        arr = np.asarray(arr, np.float64)
        if arr.ndim == 1:
            arr = arr[:, None]
        pad = np.zeros((C, arr.shape[1]), np.float64)
        pad[:arr.shape[0], :] = arr
        self.cols[name] = (self.pos, arr.shape[1])
        self.parts.append(pad)
        self.pos += arr.shape[1]

    def done(self, dt):
        return np.concatenate(self.parts, axis=1).astype(dt)


def _fold(inp):
    g = lambda k: np.asarray(inp[k], np.float64)
    hA = _merge(_taps(g('wh_m'), float(np.asarray(inp['r_m']))),
                _taps(g('wh_l'), float(np.asarray(inp['r_l']))))
    wA = _merge(_taps(g('ww_m'), float(np.asarray(inp['r_m']))),
                _taps(g('ww_l'), float(np.asarray(inp['r_l']))))
    hA[0] = hA.get(0, np.zeros(C)) + 2.0    # identity terms of m+l
    wA.setdefault(0, np.zeros(C))
    offs = sorted(d for d in set(hA) | set(wA) if d != 0)
    assert all(abs(d) <= PADH for d in offs), f"shift exceeds pad: {offs}"
    pair_ds = {d for p in PAIRS for d in p}
    assert set(offs) <= pair_ds, f"unpaired shifts: {set(offs) - pair_ds}"

    sf = g('bnf_g') / np.sqrt(g('bnf_v') + EPS)
    wfuse_t = (g('w_fuse') * sf[:, None]).T.copy()      # (Cin, Cout) lhsT
    bf = g('bnf_b') - g('bnf_m') * sf

    ds = g('dg_g') / np.sqrt(g('dg_v') + EPS)
    db = g('dg_b') - g('dg_m') * ds
    dg_wh, dg_ww = g('dg_wh'), g('dg_ww')
    anchH = {-1: ds * dg_wh[:, 0], 1: ds * dg_wh[:, 2]}
    anchW = {-1: ds * dg_ww[:, 0], 1: ds * dg_ww[:, 2]}
    cB0 = ds * (dg_wh[:, 1] + dg_ww[:, 1] + 1.0)

    cs = g('ca_g') / np.sqrt(g('ca_v') + EPS)
    cb = g('ca_b') - g('ca_m') * cs

    def lhsT_for(axis, d):
        """stationary (lhsT layout, (Cin, Cout)) for one shift term."""
        A = hA if axis == 'h' else wA
        anch = anchH if axis == 'h' else anchW
        m = np.zeros((C, C))
        if d in A:
            m = m + wfuse_t * A[d][:, None]
        if d in anch:
            m = m + np.diag(anch[d])
        return m

    # f32 consts
    pkf = _Pack()
    pkf.put('bz', bf + db)
    pkf.put('act_a', g('act_a'))
    pkf.put('zero', np.zeros(C))
    pkf.put('one', np.ones(C))
    pkf.put('cas', cs)
    pkf.put('cab', cb)
    pkf.put('caa', g('ca_a'))
    consts = pkf.done(np.float32)

    # bf16 consts: center stationary + identity (xw accumulate)
    pkb = _Pack()
    pkb.put('Acen', (wfuse_t * (hA[0] + wA[0])[:, None] + np.diag(cB0))
            * SSTAT)
    pkb.put('ident', np.eye(C))
    pkb.put('caw1b', (g('ca_w1') / float(W)).T)   # (C, 8); 1/W mean fold
    pkb.put('cawhb', g('ca_wh').T)                # (8, C)
    pkb.put('cawwb', g('ca_ww').T)
    constb = pkb.done(ml_dtypes.bfloat16)

    # fp8 consts: DoubleRow pair blocks (C, 2*C) = [slot0 | slot1].
    # All z stationaries are pre-scaled by SSTAT (power of 2) to pull the
    # small fp8 entries out of e4m3's subnormal range; the PReLU evict
    # applies scale=1/SSTAT to undo it.
    pk8 = _Pack()
    for axis in ('h', 'w'):
        for d0, d1 in PAIRS:
            blk = np.concatenate([lhsT_for(axis, d0), lhsT_for(axis, d1)], 1)
            pk8.put(f'P{axis}{d0}_{d1}', blk * SSTAT)
    const8 = pk8.done(ml_dtypes.float8_e4m3fn)

    key = (consts.shape[1], constb.shape[1], const8.shape[1])
    return consts, pkf.cols, constb, pkb.cols, const8, pk8.cols, key


def _prep_x(x):
    """Per-sample input tiles: bf16 flat, fp8 H-padded, fp8 W-padded
    chunk-transposed."""
    xb = x.astype(ml_dtypes.bfloat16).reshape(B, C, HW)
    x8 = x.astype(ml_dtypes.float8_e4m3fn)
    f8h = np.zeros((B, C, HP, W), ml_dtypes.float8_e4m3fn)
    f8h[:, :, PADH:PADH + H, :] = x8
    # x_f8w[c, k, w', j] = x[c, 4k+j, w'-PADW]
    f8w = np.zeros((B, C, NCHUNK, WP, CH), ml_dtypes.float8_e4m3fn)
    xr = x8.reshape(B, C, NCHUNK, CH, W)
    f8w[:, :, :, PADW:PADW + W, :] = np.swapaxes(xr, 3, 4)
    return (xb, f8h.reshape(B, C, HP * W), f8w.reshape(B, C, NCHUNK * WP * CH))


# -------------------------------------------------------------- graph builder
def _build(colf, colb, col8, ckf, ckb, ck8):
    nc = bacc.Bacc()
    xb_p = nc.declare_dram_parameter("xb", (C, HW), bf16, isOutput=False)
    xh_p = nc.declare_dram_parameter("xf8h", (C, HP * W), f8, isOutput=False)
    xw_p = nc.declare_dram_parameter("xf8w", (C, NCHUNK * WP * CH), f8,
                                     isOutput=False)
    cf_p = nc.declare_dram_parameter("consts", (C, ckf), f32, isOutput=False)
    cb_p = nc.declare_dram_parameter("constb", (C, ckb), bf16, isOutput=False)
    c8_p = nc.declare_dram_parameter("const8", (C, ck8), f8, isOutput=False)
    o_p = nc.declare_dram_parameter("out", (C, HW), bf16, isOutput=True)

    with tile.TileContext(nc) as tc, ExitStack() as ctx:
        big = ctx.enter_context(tc.tile_pool(name="big", bufs=1))
        tpool = ctx.enter_context(tc.tile_pool(name="tpool", bufs=3))
        psq = ctx.enter_context(tc.tile_pool(name="psq", bufs=4, space="PSUM"))
        psa = ctx.enter_context(tc.tile_pool(name="psa", bufs=1, space="PSUM"))
        pss = ctx.enter_context(tc.tile_pool(name="pss", bufs=2, space="PSUM"))

        cbt = big.tile([C, ckb], bf16, tag="cbt")
        nc.sync.dma_start(cbt[:], cb_p[:])
        c8t = big.tile([C, ck8], f8, tag="c8t")
        nc.sync.dma_start(c8t[:], c8_p[:])
        cst = big.tile([C, ckf], f32, tag="cst")
        nc.scalar.dma_start(cst[:], cf_p[:])

        def cc(name, i=0):
            p0, n = colf[name]
            assert i < n
            return cst[:, p0 + i:p0 + i + 1]

        def crf(name, rows=C):
            p0, n = colf[name]
            return cst[0:rows, p0:p0 + n]

        def cbr(name):
            p0, n = colb[name]
            return cbt[0:C, p0:p0 + n]

        def c8pair(name):
            p0, n = col8[name]
            return c8t[0:C, p0:p0 + n].rearrange("p (t m) -> p t m", t=2)

        # input tiles + chunked DMA, interleaved so chunk 0 deps land first
        xb_t = big.tile([C, HW], bf16, tag="xb")
        xh_t = big.tile([C, HP * W], f8, tag="xf8h")
        xw_t = big.tile([C, NCHUNK * WP * CH], f8, tag="xf8w")
        pre = big.tile([C, HW], bf16, tag="pre")
        pre3 = pre[:].rearrange("p (h w) -> p h w", w=W)

        # chunked input DMA: few slabs (SP seq cost ~0.6us per dma_start),
        # first slab of each tile small so chunk 0 can start early
        SLW = WP * CH                       # 640 cols per xf8w chunk
        def slabs(total, first, step):
            cuts = [0, first]
            while cuts[-1] < total:
                cuts.append(min(total, cuts[-1] + step))
            return list(zip(cuts[:-1], cuts[1:]))
        dma_order = []
        for (a, b) in slabs(NCHUNK, 4, 7):
            dma_order.append(('w', a * SLW, b * SLW))
        for (a, b) in slabs(HP // 8, 5, 5):
            dma_order.append(('h', a * 8 * W, b * 8 * W))
        for (a, b) in slabs(NBLK, 2, 4):
            dma_order.append(('b', a * BH * W, b * BH * W))
        # chunk-0 deps first: W slab (W-passes run first), then H halo
        pri = {'w': 0, 'h': 1, 'b': 2}
        dma_order.sort(key=lambda t: (t[1], pri[t[0]]))
        # split input streams across queues: each dma_start holds its
        # queue's sequencer ~0.6-2us, so one queue serializes the fill.
        # SP: fp8 tiles; Pool (SWDGE, idle early): the bf16 center tile.
        for kind, a, b in dma_order:
            t, p = {'w': (xw_t, xw_p), 'h': (xh_t, xh_p),
                    'b': (xb_t, xb_p)}[kind]
            q = nc.gpsimd if kind == 'b' else nc.sync
            q.dma_start(t[:, a:b], p[:, a:b])

        zcol = cc('zero')
        # engine warmups: ACT tables, small copies, PE clock ramp.
        # The PE warmers run on a memset scratch (no DMA dependency) so the
        # PE is busy from ~0.5us through the DMA fill — the cost model
        # resets the PE clock ramp after any idle gap.
        wsc = big.tile([C, 256], bf16, tag="wsc")
        nc.vector.memset(wsc[:], 0.0)
        wrm = big.tile([C, 4], f32, tag="wrm")
        nc.scalar.activation(wrm[:, 0:1], zcol, AF.Prelu, bias=zcol, scale=1.0,
                             alpha=cc('act_a'))
        nc.scalar.activation(wrm[:, 3:4], zcol, AF.Sigmoid, bias=zcol,
                             scale=1.0)
        nc.gpsimd.tensor_copy(wrm[:, 2:3], zcol)
        pwm = pss.tile([C, 256], f32, tag="small")
        for wi in range(10):
            nc.tensor.matmul(pwm[:], wsc[:, 0:96], wsc[:],
                             start=(wi == 0), stop=(wi == 9))

        def h_rhs(k, d0, d1):
            fullap = xh_t[:]
            return bass.AP(
                tensor=fullap.tensor,
                offset=fullap.offset + (PADH + k * CH + d0) * W,
                ap=[fullap.ap[0], [(d1 - d0) * W, 2], [1, CW]])

        def w_rhs(k, d0, d1):
            fullap = xw_t[:]
            return bass.AP(
                tensor=fullap.tensor,
                offset=fullap.offset + k * SLW + (PADW + d0) * CH,
                ap=[fullap.ap[0], [(d1 - d0) * CH, 2], [CH, W], [1, CH]])

        # xw column-sum accumulator (psum), fed by deferred ident matmuls
        xwp = psa.tile([C, CH, W], f32, tag="xwp")
        yinh = big.tile([C, H], bf16, tag="yinh")   # bf16: keeps reduce at 2x
        yinw = big.tile([C, W], bf16, tag="yinw")
        # even chunks 0-22 accumulate xw on DVE (hidden in mid-loop gaps)
        # instead of PE ident matmuls; folded once at k=26
        NDVE_XW = 14
        ywd = big.tile([C, W * NDVE_XW], bf16, tag="ywd")
        ywd3 = ywd[:].rearrange("p (w k) -> p w k", k=NDVE_XW)
        yw2a = big.tile([C, W], bf16, tag="yw2a")
        # odd chunks 1-23 accumulate xw on Pool via row-fold chains
        accw = big.tile([C, W], bf16, tag="accw")
        ah = big.tile([C, H], bf16, tag="ah")
        aw = big.tile([C, W], bf16, tag="aw")
        y2 = big.tile([8, 2 * H], bf16, tag="y2")
        stage_t = [big.tile([C, BH * W], bf16, tag=f"st{b}", name=f"st{b}")
                   for b in range(NBLK)]

        deferred = []   # (chunk, pre-slice AP) for xw ident matmuls
        gcnt = [0]
        NPE_XW = NCHUNK - 2 * NDVE_XW

        def emit_xw(k, sl):
            nc.tensor.matmul(xwp[:], cbr('ident'), sl,
                             start=(gcnt[0] == 0),
                             stop=(gcnt[0] == NPE_XW - 1))
            gcnt[0] += 1

        # hoist the first center passes: they only need the first xb
        # slab, so they fill the PE while the fp8 halo slabs stream in
        pk_hoist = []
        for k in range(3):
            pk = psq.tile([C, CH, W], f32, tag="pk", name=f"pkh{k}")
            pk_hoist.append(pk)
            nc.tensor.matmul(pk[:].rearrange("p r w -> p (r w)"), cbr('Acen'),
                             xb_t[:, k * CW:(k + 1) * CW],
                             start=True, stop=False)

        for k in range(NCHUNK):
            if k < 3:
                pk = pk_hoist[k]
            else:
                pk = psq.tile([C, CH, W], f32, tag="pk")
            pkf = pk[:].rearrange("p r w -> p (r w)")
            if k >= 3:
                nc.tensor.matmul(pkf, cbr('Acen'),
                                 xb_t[:, k * CW:(k + 1) * CW],
                                 start=True, stop=False)
            pkt = pk[:].rearrange("p r w -> p w r")
            for d0, d1 in PAIRS:
                nc.tensor.matmul(pkt, c8pair(f'Pw{d0}_{d1}'), w_rhs(k, d0, d1),
                                 start=False, stop=False, perf_mode=DR)
            for i, (d0, d1) in enumerate(PAIRS):
                nc.tensor.matmul(pkf, c8pair(f'Ph{d0}_{d1}'), h_rhs(k, d0, d1),
                                 start=False, stop=(i == len(PAIRS) - 1),
                                 perf_mode=DR)
            # evict: pre = prelu(z/SSTAT + bz)
            nc.scalar.activation(pre[:, k * CW:(k + 1) * CW], pkf,
                                 AF.Prelu, bias=cc('bz'), scale=1.0 / SSTAT,
                                 alpha=cc('act_a'))
            sl = pre3[:, k * CH:(k + 1) * CH, :]
            if k >= NCHUNK - 2:
                # last block's xh as per-chunk reduces: the chunk-30 half
                # runs off-critical; only 593ns rides the tail
                with nc.allow_low_precision(reason="xh sums feed gates"):
                    nc.vector.tensor_reduce(yinh[:, k * CH:(k + 1) * CH],
                                            sl, axis=mybir.AxisListType.X,
                                            op=ALU.add)
            if k % 2 == 0 and k < 2 * NDVE_XW:
                with nc.allow_low_precision(reason="xw partial sums feed "
                                            "sigmoid gates"):
                    nc.vector.tensor_reduce(
                        ywd3[:, :, k // 2:k // 2 + 1].squeeze(2),
                        sl.rearrange("p h w -> p w h"),
                        axis=mybir.AxisListType.X, op=ALU.add)
            elif k % 2 == 1 and k < 2 * NDVE_XW:
                t1 = tpool.tile([C, 2 * W], bf16, tag="xf")
                t13 = t1[:].rearrange("p (h w) -> p h w", w=W)
                nc.gpsimd.tensor_tensor(t13, sl[:, 0:2, :], sl[:, 2:4, :],
                                        op=ALU.add)
                t1a = t13[:, 0:1, :].squeeze(1)
                t1b = t13[:, 1:2, :].squeeze(1)
                if k == 1:
                    nc.gpsimd.tensor_tensor(accw[:], t1a, t1b, op=ALU.add)
                else:
                    t2 = tpool.tile([C, W], bf16, tag="xf2")
                    nc.gpsimd.tensor_tensor(t2[:], t1a, t1b, op=ALU.add)
                    nc.gpsimd.tensor_tensor(accw[:], accw[:], t2[:],
                                            op=ALU.add)
            else:
                deferred.append((k, sl))
                if len(deferred) >= 3:
                    emit_xw(*deferred.pop(0))
            if k == 2 * NDVE_XW + 2:
                with nc.allow_low_precision(reason="xw partials"):
                    nc.vector.tensor_reduce(yw2a[:], ywd3,
                                            axis=mybir.AxisListType.X,
                                            op=ALU.add)

            if k % 2 == 1:
                # xh row sums: fold W in half on Pool (522ns), half again on
                # DVE (193ns), small reduce on DVE (327ns) — a direct DVE
                # reduce is 1127ns (tensor_reduce never gets a perf mode).
                # The last block reduces directly on DVE: the fold chain's
                # Pool hop would sit on the critical tail.
                b = k // 2   # finished block
                r0, r1 = b * BH, (b + 1) * BH
                with nc.allow_low_precision(reason="xh sums feed sigmoid "
                                            "gates"):
                    if b == NBLK - 1:
                        pass   # handled per-chunk below
                    else:
                        fold1 = tpool.tile([C, BH * (W // 2)], bf16,
                                           tag="f1")
                        f13 = fold1[:].rearrange("p (h w) -> p h w",
                                                 w=W // 2)
                        nc.gpsimd.tensor_tensor(f13,
                                                pre3[:, r0:r1, 0:W // 2],
                                                pre3[:, r0:r1, W // 2:W],
                                                op=ALU.add)
                        fold2 = tpool.tile([C, BH * (W // 4)], bf16,
                                           tag="f2")
                        f23 = fold2[:].rearrange("p (h w) -> p h w",
                                                 w=W // 4)
                        nc.vector.tensor_tensor(f23, f13[:, :, 0:W // 4],
                                                f13[:, :, W // 4:W // 2],
                                                op=ALU.add)
                        nc.vector.tensor_reduce(yinh[:, r0:r1], f23,
                                                axis=mybir.AxisListType.X,
                                                op=ALU.add)
            gblk = NBLK // NGRP              # blocks per gate group
            if k % (2 * gblk) == 2 * gblk - 1:
                gg = k // (2 * gblk)         # finished group
                b0g, b1g = gg * gblk, (gg + 1) * gblk
                c0, c1 = b0g * BH, b1g * BH   # h cols
                y1p = pss.tile([8, c1 - c0], f32, tag="small")
                nc.tensor.matmul(y1p[:], cbr('caw1b')[:, 0:8], yinh[:, c0:c1],
                                 start=True, stop=True)
                nc.scalar.activation(y2[:, c0:c1], y1p[:], AF.Prelu,
                                     bias=cc('cab')[0:8, :],
                                     scale=cc('cas')[0:8, :],
                                     alpha=cc('caa')[0:8, :])
                ahp = pss.tile([C, c1 - c0], f32, tag="small")
                nc.tensor.matmul(ahp[:], cbr('cawhb')[0:8, :], y2[:, c0:c1],
                                 start=True, stop=True)
                nc.scalar.activation(ah[:, c0:c1], ahp[:], AF.Sigmoid,
                                     bias=zcol, scale=1.0)
                # stage: o = pre * ah (bcast w) into separate tiles so the
                # xw matmuls keep reading a clean pre
                for b in range(b0g, b1g):
                    r0, r1 = b * BH, (b + 1) * BH
                    ah_b = ah[:, r0:r1].unsqueeze(2).broadcast_to((C, BH, W))
                    o3 = stage_t[b][:].rearrange("p (h w) -> p h w", w=W)
                    # ah bcast has stride-0 innermost -> 1x on DVE, so Pool
                    # (853ns) takes most stages
                    eng = nc.vector if b % 4 == 3 else nc.gpsimd
                    eng.tensor_tensor(o3, pre3[:, r0:r1, :], ah_b,
                                      op=ALU.mult)

        for item in deferred:
            emit_xw(*item)

        # aw tail: fold xw psum, add the DVE partial, tiny chain
        with nc.allow_low_precision(reason="xw sums feed sigmoid gates"):
            nc.vector.tensor_reduce(yinw[:],
                                    xwp[:].rearrange("p j w -> p w j"),
                                    axis=mybir.AxisListType.X, op=ALU.add)
        nc.vector.tensor_tensor(yinw[:], yinw[:], yw2a[:], op=ALU.add)
        nc.vector.tensor_tensor(yinw[:], yinw[:], accw[:], op=ALU.add)
        y1w = pss.tile([8, H], f32, tag="small")
        nc.tensor.matmul(y1w[:], cbr('caw1b')[:, 0:8], yinw[:],
                         start=True, stop=True)
        nc.scalar.activation(y2[:, H:2 * H], y1w[:], AF.Prelu,
                             bias=cc('cab')[0:8, :], scale=cc('cas')[0:8, :],
                             alpha=cc('caa')[0:8, :])
        awp = pss.tile([C, W], f32, tag="small")
        nc.tensor.matmul(awp[:], cbr('cawwb')[0:8, :], y2[:, H:2 * H],
                         start=True, stop=True)
        nc.scalar.activation(aw[:], awp[:], AF.Sigmoid, bias=zcol, scale=1.0)

        # finals: out = stage * aw (bcast h; packed innermost everywhere ->
        # STT can hit the DVE 4x mode). Out-DMAs alternate idle SP/ACT.
        aw_b = aw[:].unsqueeze(1).broadcast_to((C, BH, W))
        for b in range(NBLK):
            r0, r1 = b * BH, (b + 1) * BH
            o3 = stage_t[b][:].rearrange("p (h w) -> p h w", w=W)
            eng = nc.gpsimd if b % 16 in (2, 5, 8, 11, 14) else nc.vector
            eng.tensor_tensor(o3, o3, aw_b, op=ALU.mult)
            dq = nc.sync if b % 2 == 0 else nc.scalar
            dq.dma_start(o_p[:, r0 * W:r1 * W], stage_t[b][:])

    nc.compile()
    return nc


def _get_graph(key, colf, colb, col8, ckf, ckb, ck8):
    if key not in _GRAPH_CACHE:
        _GRAPH_CACHE[key] = _build(colf, colb, col8, ckf, ckb, ck8)
    return _GRAPH_CACHE[key]


# ------------------------------------------------------------------ interface
def _run(inputs, trace=False):
    x = np.ascontiguousarray(np.asarray(inputs['x'], np.float32))
    assert x.shape == (B, C, H, W)
    consts, colf, constb, colb, const8, col8, key = _fold(inputs)
    nc = _get_graph(key, colf, colb, col8,
                    consts.shape[1], constb.shape[1], const8.shape[1])
    xb, f8h, f8w = _prep_x(x)
    in_maps = []
    for i in range(N_CORES):
        in_maps.append({'xb': np.ascontiguousarray(xb[i]),
                        'xf8h': np.ascontiguousarray(f8h[i]),
                        'xf8w': np.ascontiguousarray(f8w[i]),
                        'consts': consts, 'constb': constb,
                        'const8': const8})
    res = run_bass_kernel_spmd(nc, in_maps, list(range(N_CORES)), trace=trace)
    out = np.stack([res.results[i]['out'].astype(np.float32).reshape(C, H, W)
                    for i in range(N_CORES)], axis=0)
    return out, res


def kernel(**inputs):
    out, _ = _run(inputs, trace=False)
    return out
